# revision 11
# baseline (speedup 1.0000x reference)
"""LIF spike-train kernel for Trainium2 (Bass/Tile), data-parallel over 8 cores.

Reference semantics (T=4, tau=0.5, thresh=1.0), per element:
    mem = 0
    for t in range(4):
        mem = mem*0.5 + x[t]
        s[t] = (mem - 1 >= 0)
        mem = mem - s[t]

x: [T*B, C, H, W] = [256, 128, 32, 32] f32, viewed as [4, 64, 128, 1024].
Batch dim (64) is sharded 8-ways; each core streams [4, 8, 128, 1024],
flattened to x,y: [T, 128, F] (F = 8192).

Scheme "psum" (PE/PSUM membrane offload; successor to the 75.4us SignFlow):
  The kernel is DMA-port-bound at ~49us of port time (16.8MB loads +
  4.2MB fp8 stores at ~425GB/s measured) + ~8.7us fixed framework
  preamble, so the goal is to get every ENGINE under that line and keep
  the load queue saturated.  The old scheme had DVE at 55-58us busy
  (6 fp32 2-src stt ops per 2048-chunk); fp32 tensor_tensor is stuck at
  1x (no DVE perf mode), so the reset op is moved off the DVE entirely:

  per chunk of CW=1024, per step t<3 (membrane v_t = u_t - s_t in PSUM):
    cmp_t:  s_t = is_ge(u_t, 1.0) -> fp8 {0, 0x38}     DVE 1-src 2x (594ns)
    reset:  B = 0.5*u_t   (Act Copy scale=0.5 -> PSUM, exact)   (~1.1us)
            B += -0.5*s_t (PE matmul, bf16 -0.5*I @ fp8 s, exact) (~0.4us)
    integ:  u_{t+1} = stt(B, +, x_{t+1}) -> SBUF       DVE 2-src 1x (1.2us)
    t=3:    sign(u_3 - c) on Act (bias AP), byte {0xB8,0,0x38}
  Rounding matches the reference exactly: 0.5*u and 0.5*s are exact,
  PSUM accumulate gives fl(0.5u - 0.5s) = 0.5*v exactly, and the stt is
  the single rounding fl(0.5v + x) that the reference performs.
  Projected busy: DVE ~43us, Act ~37us, PE ~10us -- all under the port
  line, vs DVE 55-58us before.

  KEY HW FACT (micro-validated in micro_psum.py on HW): PSUM has a
  per-element has_written bit that only PE matmuls set; a matmul with
  start=False OVERWRITES (rather than accumulates onto) Act-written
  data wherever the bit is clear.  This is what corrupted the earlier
  PE attempts.  Fix: one dummy matmul(start=True) per PSUM bank in the
  preamble sets the bits for the whole bank; they are never cleared
  again, so Act-write -> matmul(start=False) accumulates correctly for
  every later generation.  Verified bit-exact over 3 generations.

  Layout: 8 chunks of 1024 processed chunk-major (full T chain per
  chunk) in 4 pairs; loads are [128,2048] per (t, pair) on the sync
  queue (8KB/partition rows, ~420GB/s); each pair's 4 spike tiles are
  stored as [128,2048] fp8, emitted AFTER the next pair's loads so the
  store's sem wait never blocks loads in the in-order sync stream.
  PSUM: 4 B-tags x 2 banks (bufs=1, fresh generation per step) = all 8
  banks; chunk j uses tag j%4 => 4 chunks in flight.
  Host decodes spike := byte == 0x38.

All spike decisions are bit-exact vs the fp32 reference (rel err 0.0).
"""

import os
import sys

sys.path.insert(0, "/opt/trn_rl_repo")

import numpy as np

T = 4
B = 64
C = 128
HW = 1024
NCORES = 8
BLOC = B // NCORES  # 8 batch elements per core
F = BLOC * C * HW // 128  # 8192 flat free width per t-block
C_THRESH = float(np.nextafter(np.float32(1.0), np.float32(0.0)))

LAST_EXEC_NS = None
LAST_TRACE = None

_CACHE = {}


def _build_psum():
    """PE/PSUM membrane scheme: DVE cmp+integrate, Act+PE reset."""
    import concourse.bacc as bacc
    import concourse.mybir as mybir
    from concourse import tile

    f32 = mybir.dt.float32
    fp8 = mybir.dt.float8e4
    bf16 = mybir.dt.bfloat16
    A = mybir.AluOpType
    AF = mybir.ActivationFunctionType

    CW = 1024  # compute chunk width (B tile = 2 PSUM banks)
    PW = 2048  # load/store pair width
    NP = F // PW  # 4 pairs
    NBT = int(os.environ.get("LIF_NBT", "4"))  # B tags (chunks in flight)

    xbufs = int(os.environ.get("LIF_XBUFS", "10"))
    ubufs = int(os.environ.get("LIF_UBUFS", "6"))
    t8bufs = int(os.environ.get("LIF_T8BUFS", "2"))
    st_name = os.environ.get("LIF_STORE_ENG", "gpsimd")
    x0split = int(os.environ.get("LIF_X0SPLIT", "4"))  # pair-0 t0 load pieces
    tailsub = int(os.environ.get("LIF_TAILSUB", "512"))  # last-pair t3 grain

    nc = bacc.Bacc("TRN2", target_bir_lowering=False, debug=False, num_devices=NCORES)
    x = nc.dram_tensor("x", [T, 128, F], f32, kind="ExternalInput").ap()
    y = nc.dram_tensor("y", [T, 128, F], fp8, kind="ExternalOutput").ap()
    wm = nc.dram_tensor("w", [128, 128], bf16, kind="ExternalInput").ap()

    with tile.TileContext(nc) as tc:
        with tc.tile_pool(name="p", bufs=xbufs) as pool, tc.psum_pool(
            name="ps", bufs=1
        ) as ppool:
            st = {"sync": nc.sync, "scalar": nc.scalar, "gpsimd": nc.gpsimd}[st_name]

            # wt on the scalar (HWDGE) queue: keeps the sync queue pure-loads
            wt = pool.tile([128, 128], bf16, tag="wt", bufs=1)
            nc.scalar.dma_start(out=wt, in_=wm)
            bias = pool.tile([128, 1], f32, tag="bias", bufs=1)
            warm = pool.tile([128, 1], fp8, tag="warm", bufs=1)
            dummy = pool.tile([128, 512], bf16, tag="dummy", bufs=1)
            nc.vector.memset(bias, -C_THRESH)
            nc.vector.memset(dummy, 0.0)
            # pull the ACT table load into the preamble
            nc.scalar.activation(warm, bias, AF.Sign, bias=bias)

            # PSUM has_written warmup: one start=True matmul per bank.
            # Keep a handle per tag; generations after this reuse the banks.
            for k in range(NBT):
                Bw = ppool.tile([128, CW], f32, tag=f"B{k}", bufs=1, name=f"Bw{k}")
                for h in range(CW // 512):
                    nc.tensor.matmul(
                        Bw[:, h * 512 : (h + 1) * 512],
                        wt,
                        dummy,
                        start=True,
                        stop=True,
                    )

            # ---- software-pipelined emission: chunk j runs step t at
            # round r = j + t, so 4 chunks (= 4 B tags) are in flight and
            # the per-step Act->PE->DVE chain latency is hidden behind the
            # other chunks' DVE work.  The whole input (16 MB/core = 128KB
            # per partition) is SBUF-resident: every x tile gets its own
            # tag so load triggers never carry compute-gated WAR waits --
            # the sync queue is pure loads, streaming at port rate.
            # Stores ride the idle gpsimd SWDGE queue, one round behind.
            NCH = F // CW  # 8 chunks
            xts = {}  # pair -> [xt per t]
            t8s = {}  # pair -> [t8 per t]
            Bs = {}  # chunk -> current B psum tile
            store_q = []

            # loads in DIAGONAL (consumption) order: tile (p, t) is first
            # read at round 2p+t by chunk 2p, so emit loads sorted by that
            # round -- pair-major order would land every t>=2 tile a few us
            # after its consumer and stall the DVE once per round.
            for p in range(NP):
                xts[p] = [
                    pool.tile(
                        [128, PW], f32, tag=f"x{p}_{t}", bufs=1, name=f"x_{p}_{t}"
                    )
                    for t in range(T)
                ]
                t8s[p] = [
                    pool.tile(
                        [128, PW], fp8, tag=f"t8_{p}_{t}", bufs=1,
                        name=f"t8_{p}_{t}",
                    )
                    for t in range(T)
                ]
            for p, t in sorted(
                ((p, t) for p in range(NP) for t in range(T)),
                key=lambda pt: (2 * pt[0] + pt[1], pt[1]),
            ):
                xt = xts[p][t]
                if p == 0 and t == 0 and x0split > 1:
                    wsub = PW // x0split
                    for k in range(x0split):
                        nc.sync.dma_start(
                            out=xt[:, k * wsub : (k + 1) * wsub],
                            in_=x[0][:, k * wsub : (k + 1) * wsub],
                        )
                else:
                    nc.sync.dma_start(out=xt, in_=x[t][:, p * PW : (p + 1) * PW])

            def emit_step(j, t):
                p, ci = j // 2, j % 2
                sl = slice(ci * CW, (ci + 1) * CW)
                if t == 0:
                    u = xts[p][0][:, sl]
                else:
                    u = pool.tile(
                        [128, CW], f32, tag="u", bufs=ubufs, name=f"u_{j}_{t}"
                    )
                    nc.vector.scalar_tensor_tensor(
                        u, Bs[j], 0.0, xts[p][t][:, sl], A.add, A.add
                    )
                if t < T - 1:
                    # reset setup: B = 0.5*u (Act) then B += -0.5*s (PE).
                    # Copy emitted before cmp: both only need u, run on
                    # different engines concurrently.
                    Bn = ppool.tile(
                        [128, CW], f32, tag=f"B{j % NBT}", bufs=1, name=f"B{j}_{t}"
                    )
                    nc.scalar.activation(Bn, u, AF.Copy, 0.0, 0.5)
                    nc.vector.tensor_scalar(t8s[p][t][:, sl], u, 1.0, None, A.is_ge)
                    for h in range(CW // 512):
                        hs = slice(h * 512, (h + 1) * 512)
                        nc.tensor.matmul(
                            Bn[:, hs],
                            wt,
                            t8s[p][t][:, sl][:, hs],
                            start=False,
                            stop=True,
                            skip_group_check=True,
                        )
                    Bs[j] = Bn
                elif j == NCH - 1:
                    # final chunk: fine-grained sign+store tail
                    nq = max(CW // tailsub, 1)
                    q = CW // nq
                    for k in range(nq):
                        usub = slice(k * q, (k + 1) * q)
                        ysub = slice(
                            p * PW + ci * CW + k * q, p * PW + ci * CW + (k + 1) * q
                        )
                        nc.scalar.activation(
                            t8s[p][3][:, sl][:, usub], u[:, usub], AF.Sign,
                            bias=bias,
                        )
                        nc.sync.dma_start(
                            out=y[3][:, ysub], in_=t8s[p][3][:, sl][:, usub]
                        )
                else:
                    nc.scalar.activation(t8s[p][3][:, sl], u, AF.Sign, bias=bias)

            for r in range(NCH + T - 1):
                # flush due stores (t8 of (t, pair p) complete after round
                # 2p+1+t) on the store engine's own queue
                while store_q and store_q[0][0] <= r:
                    _, t, p = store_q.pop(0)
                    if not (t == 3 and p == NP - 1):
                        st.dma_start(
                            out=y[t][:, p * PW : (p + 1) * PW], in_=t8s[p][t]
                        )
                    else:
                        # chunk NCH-1's t3 slice already stored fine-grained
                        st.dma_start(
                            out=y[3][:, p * PW : p * PW + CW], in_=t8s[p][3][:, 0:CW]
                        )
                for j in range(NCH):
                    t = r - j
                    if 0 <= t < T:
                        emit_step(j, t)
                        if j % 2 == 1:  # pair's second chunk passed step t
                            store_q.append((r + 1, t, j // 2))
            while store_q:
                _, t, p = store_q.pop(0)
                if not (t == 3 and p == NP - 1):
                    st.dma_start(out=y[t][:, p * PW : (p + 1) * PW], in_=t8s[p][t])
                else:
                    st.dma_start(
                        out=y[3][:, p * PW : p * PW + CW], in_=t8s[p][3][:, 0:CW]
                    )

    nc.compile()
    return nc


def _build_sign():
    """Fallback: previous SignFlow scheme (~75.4us). See git history of the
    docstring for details; kept for A/B via LIF_SCHEME=sign."""
    import concourse.bacc as bacc
    import concourse.mybir as mybir
    from concourse import tile

    f32 = mybir.dt.float32
    i8 = mybir.dt.int8
    A = mybir.AluOpType
    AF = mybir.ActivationFunctionType

    W = min(int(os.environ.get("LIF_W", "2048")), F)
    CW = min(int(os.environ.get("LIF_CW", str(W))), W)
    NCH = F // W
    SUB = W // CW
    NCC = F // CW
    assert F % W == 0 and W % CW == 0

    nc = bacc.Bacc("TRN2", target_bir_lowering=False, debug=False, num_devices=NCORES)
    x = nc.dram_tensor("x", [T, 128, F], f32, kind="ExternalInput").ap()
    y = nc.dram_tensor("y", [T, 128, F], i8, kind="ExternalOutput").ap()

    xbufs = int(os.environ.get("LIF_XBUFS", "6"))
    ubufs = int(os.environ.get("LIF_UBUFS", "6"))
    tbufs = int(os.environ.get("LIF_TBUFS", "2"))

    with tile.TileContext(nc) as tc:
        with tc.tile_pool(name="p", bufs=xbufs) as pool:
            bias = pool.tile([128, 1], f32, tag="bias", bufs=1)
            warm = pool.tile([128, 1], i8, tag="warm", bufs=1)
            nc.vector.memset(bias, -C_THRESH)
            nc.scalar.activation(warm, bias, AF.Sign, bias=bias)

            W0 = min(int(os.environ.get("LIF_W0", str(CW))), CW)
            nvs = {}
            store_pending = None
            for t in range(T):
                xs = {}
                if t == 0 and W0 < CW:
                    t8 = pool.tile([128, F], i8, tag="t8", bufs=tbufs)
                    for j in range(NCC):
                        nvs[j] = pool.tile(
                            [128, CW], f32, tag=f"nv{j}", bufs=2, name=f"nv0_{j}"
                        )
                    for k in range(F // W0):
                        xt = pool.tile([128, W0], f32, tag="x0", bufs=6)
                        nc.sync.dma_start(out=xt, in_=x[0][:, k * W0 : (k + 1) * W0])
                        sl = slice(k * W0, (k + 1) * W0)
                        nc.scalar.activation(t8[:, sl], xt, AF.Sign, bias=bias)
                        j = (k * W0) // CW
                        nsub = slice(k * W0 - j * CW, (k + 1) * W0 - j * CW)
                        nc.vector.scalar_tensor_tensor(
                            nvs[j][:, nsub], t8[:, sl], 0.0, xt, A.max, A.subtract
                        )
                    nc.sync.dma_start(out=y[0], in_=t8)
                    continue
                for i in range(NCH):
                    xt = pool.tile([128, W], f32, tag="x")
                    nc.sync.dma_start(out=xt, in_=x[t][:, i * W : (i + 1) * W])
                    xs[i] = xt

                if store_pending is not None:
                    pt, pt8 = store_pending
                    nc.sync.dma_start(out=y[pt], in_=pt8)
                    store_pending = None

                t8 = pool.tile([128, F], i8, tag="t8", bufs=tbufs)
                pending = None

                def emit_reset(j, u):
                    sl = slice(j * CW, (j + 1) * CW)
                    nv = pool.tile([128, CW], f32, tag=f"nv{j}", bufs=2)
                    nc.vector.scalar_tensor_tensor(
                        nv, t8[:, sl], 0.0, u, A.max, A.subtract
                    )
                    nvs[j] = nv

                for j in range(NCC):
                    sl = slice(j * CW, (j + 1) * CW)
                    xsl = xs[j // SUB][:, (j % SUB) * CW : (j % SUB + 1) * CW]
                    if t == T - 1 and j == NCC - 1:
                        u = pool.tile([128, CW], f32, tag="u", bufs=ubufs)
                        nq = 4
                        q = CW // nq
                        for k in range(nq):
                            usub = slice(k * q, (k + 1) * q)
                            ysub = slice(j * CW + k * q, j * CW + (k + 1) * q)
                            nc.vector.scalar_tensor_tensor(
                                u[:, usub], nvs[j][:, usub], -0.5,
                                xsl[:, usub], A.mult, A.add,
                            )
                            nc.scalar.activation(
                                t8[:, ysub], u[:, usub], AF.Sign, bias=bias
                            )
                            nc.sync.dma_start(out=y[t][:, ysub], in_=t8[:, ysub])
                        continue
                    if t == 0:
                        u = xsl
                    else:
                        u = pool.tile([128, CW], f32, tag="u", bufs=ubufs)
                        nc.vector.scalar_tensor_tensor(
                            u, nvs[j], -0.5, xsl, A.mult, A.add
                        )
                    if t == 0 and j < 2:
                        nc.vector.tensor_scalar(t8[:, sl], u, 1.0, None, A.is_ge)
                    else:
                        nc.scalar.activation(t8[:, sl], u, AF.Sign, bias=bias)
                    if t < T - 1:
                        if pending is not None:
                            emit_reset(*pending)
                        pending = (j, u)
                    else:
                        nc.sync.dma_start(out=y[t][:, sl], in_=t8[:, sl])
                if pending is not None:
                    emit_reset(*pending)

                if t < T - 1:
                    store_pending = (t, t8)
            if store_pending is not None:
                pt, pt8 = store_pending
                nc.sync.dma_start(out=y[pt], in_=pt8)

    nc.compile()
    return nc


def _get_nc():
    if "nc" not in _CACHE:
        scheme = os.environ.get("LIF_SCHEME", "psum")
        _CACHE["scheme"] = scheme
        _CACHE["nc"] = _build_sign() if scheme == "sign" else _build_psum()
    return _CACHE["nc"]


def kernel(x: np.ndarray) -> np.ndarray:
    global LAST_EXEC_NS, LAST_TRACE
    from concourse.bass_utils import run_bass_kernel_spmd

    x = np.ascontiguousarray(np.asarray(x), dtype=np.float32)
    assert x.shape == (T * B, C, 32, 32), x.shape
    xv = x.reshape(T, B, C, HW)

    nc = _get_nc()
    scheme = _CACHE.get("scheme", "psum")

    wI = None
    if scheme != "sign":
        import ml_dtypes

        wI = (np.eye(128, dtype=np.float32) * -0.5).astype(ml_dtypes.bfloat16)

    in_maps = []
    for m in range(NCORES):
        shard = np.ascontiguousarray(xv[:, m * BLOC : (m + 1) * BLOC]).reshape(
            T, 128, F
        )
        im = {"x": shard}
        if wI is not None:
            im["w"] = wI
        in_maps.append(im)

    trace = os.environ.get("LIF_TRACE") == "1"
    res = run_bass_kernel_spmd(nc, in_maps, core_ids=list(range(NCORES)), trace=trace)
    LAST_EXEC_NS = res.exec_time_ns
    if res.instructions_and_trace is not None:
        LAST_TRACE = res.instructions_and_trace[1]

    out = np.empty((T, B, C, HW), dtype=np.float32)
    for m in range(NCORES):
        raw = np.asarray(res.results[m]["y"])
        if scheme == "sign":
            sp = raw.view(np.int8) == 1
        else:
            # fp8e4 bytes: 1.0 = 0x38 (spike); 0x00 / 0xB8 (-1.0) = no spike
            sp = raw.view(np.uint8) == 0x38
        out[:, m * BLOC : (m + 1) * BLOC] = sp.astype(np.float32).reshape(
            T, BLOC, C, HW
        )
    return out.reshape(T * B, C, 32, 32)


# revision 12
# speedup vs baseline: 1.1559x; 1.1559x over previous
"""LIF spike-train kernel for Trainium2 (Bass/Tile), data-parallel over 8 cores.

Reference semantics (T=4, tau=0.5, thresh=1.0), per element:
    mem = 0
    for t in range(4):
        mem = mem*0.5 + x[t]
        s[t] = (mem - 1 >= 0)
        mem = mem - s[t]

x: [T*B, C, H, W] = [256, 128, 32, 32] f32, viewed as [4, 64, 128, 1024].
Batch dim (64) is sharded 8-ways; each core streams [4, 8, 128, 1024],
flattened to x,y: [T, 128, F] (F = 8192).

Scheme "psum" (PE/PSUM membrane offload; successor to the 75.4us SignFlow):
  The kernel is DMA-port-bound at ~49us of port time (16.8MB loads +
  4.2MB fp8 stores at ~425GB/s measured) + ~8.7us fixed framework
  preamble, so the goal is to get every ENGINE under that line and keep
  the load queue saturated.  The old scheme had DVE at 55-58us busy
  (6 fp32 2-src stt ops per 2048-chunk); fp32 tensor_tensor is stuck at
  1x (no DVE perf mode), so the reset op is moved off the DVE entirely:

  per chunk of CW=1024, per step t<3 (membrane v_t = u_t - s_t in PSUM):
    cmp_t:  s_t = is_ge(u_t, 1.0) -> fp8 {0, 0x38}     DVE 1-src 2x (594ns)
    reset:  B = 0.5*u_t   (Act Copy scale=0.5 -> PSUM, exact)   (~1.1us)
            B += -0.5*s_t (PE matmul, bf16 -0.5*I @ fp8 s, exact) (~0.4us)
    integ:  u_{t+1} = stt(B, +, x_{t+1}) -> SBUF       DVE 2-src 1x (1.2us)
    t=3:    sign(u_3 - c) on Act (bias AP), byte {0xB8,0,0x38}
  Rounding matches the reference exactly: 0.5*u and 0.5*s are exact,
  PSUM accumulate gives fl(0.5u - 0.5s) = 0.5*v exactly, and the stt is
  the single rounding fl(0.5v + x) that the reference performs.
  Projected busy: DVE ~43us, Act ~37us, PE ~10us -- all under the port
  line, vs DVE 55-58us before.

  KEY HW FACT (micro-validated in micro_psum.py on HW): PSUM has a
  per-element has_written bit that only PE matmuls set; a matmul with
  start=False OVERWRITES (rather than accumulates onto) Act-written
  data wherever the bit is clear.  This is what corrupted the earlier
  PE attempts.  Fix: one dummy matmul(start=True) per PSUM bank in the
  preamble sets the bits for the whole bank; they are never cleared
  again, so Act-write -> matmul(start=False) accumulates correctly for
  every later generation.  Verified bit-exact over 3 generations.

  Layout: 8 chunks of 1024 processed chunk-major (full T chain per
  chunk) in 4 pairs; loads are [128,2048] per (t, pair) on the sync
  queue (8KB/partition rows, ~420GB/s); each pair's 4 spike tiles are
  stored as [128,2048] fp8, emitted AFTER the next pair's loads so the
  store's sem wait never blocks loads in the in-order sync stream.
  PSUM: 4 B-tags x 2 banks (bufs=1, fresh generation per step) = all 8
  banks; chunk j uses tag j%4 => 4 chunks in flight.
  Host decodes spike := byte == 0x38.

All spike decisions are bit-exact vs the fp32 reference (rel err 0.0).
"""

import os
import sys

sys.path.insert(0, "/opt/trn_rl_repo")

import numpy as np

T = 4
B = 64
C = 128
HW = 1024
NCORES = 8
BLOC = B // NCORES  # 8 batch elements per core
F = BLOC * C * HW // 128  # 8192 flat free width per t-block
C_THRESH = float(np.nextafter(np.float32(1.0), np.float32(0.0)))

LAST_EXEC_NS = None
LAST_TRACE = None

_CACHE = {}


def _build_psum():
    """PE/PSUM membrane scheme: DVE cmp+integrate, Act+PE reset."""
    import concourse.bacc as bacc
    import concourse.mybir as mybir
    from concourse import tile

    f32 = mybir.dt.float32
    fp8 = mybir.dt.float8e4
    bf16 = mybir.dt.bfloat16
    A = mybir.AluOpType
    AF = mybir.ActivationFunctionType

    CW = 1024  # compute chunk width (B tile = 2 PSUM banks)
    PW = 2048  # load/store pair width
    NP = F // PW  # 4 pairs
    NBT = int(os.environ.get("LIF_NBT", "4"))  # B tags (chunks in flight)

    xbufs = int(os.environ.get("LIF_XBUFS", "10"))
    ubufs = int(os.environ.get("LIF_UBUFS", "6"))
    t8bufs = int(os.environ.get("LIF_T8BUFS", "2"))
    st_name = os.environ.get("LIF_STORE_ENG", "sync")
    x0split = int(os.environ.get("LIF_X0SPLIT", "4"))  # pair-0 t0 load pieces
    tailsub = int(os.environ.get("LIF_TAILSUB", "512"))  # last-pair t3 grain

    nc = bacc.Bacc("TRN2", target_bir_lowering=False, debug=False, num_devices=NCORES)
    x = nc.dram_tensor("x", [T, 128, F], f32, kind="ExternalInput").ap()
    y = nc.dram_tensor("y", [T, 128, F], fp8, kind="ExternalOutput").ap()
    wm = nc.dram_tensor("w", [128, 128], bf16, kind="ExternalInput").ap()

    with tile.TileContext(nc) as tc:
        with tc.tile_pool(name="p", bufs=xbufs) as pool, tc.psum_pool(
            name="ps", bufs=1
        ) as ppool:
            st = {"sync": nc.sync, "scalar": nc.scalar, "gpsimd": nc.gpsimd}[st_name]

            # wt on the scalar (HWDGE) queue: keeps the sync queue pure-loads
            wt = pool.tile([128, 128], bf16, tag="wt", bufs=1)
            nc.scalar.dma_start(out=wt, in_=wm)
            bias = pool.tile([128, 1], f32, tag="bias", bufs=1)
            warm = pool.tile([128, 1], fp8, tag="warm", bufs=1)
            dummy = pool.tile([128, 512], bf16, tag="dummy", bufs=1)
            nc.vector.memset(bias, -C_THRESH)
            nc.vector.memset(dummy, 0.0)
            # pull the ACT table load into the preamble
            nc.scalar.activation(warm, bias, AF.Sign, bias=bias)

            # PSUM has_written warmup: one start=True matmul per bank.
            # Keep a handle per tag; generations after this reuse the banks.
            for k in range(NBT):
                Bw = ppool.tile([128, CW], f32, tag=f"B{k}", bufs=1, name=f"Bw{k}")
                for h in range(CW // 512):
                    nc.tensor.matmul(
                        Bw[:, h * 512 : (h + 1) * 512],
                        wt,
                        dummy,
                        start=True,
                        stop=True,
                    )

            # ---- software-pipelined emission: chunk j runs step t at
            # round r = j + t, so 4 chunks (= 4 B tags) are in flight and
            # the per-step Act->PE->DVE chain latency is hidden behind the
            # other chunks' DVE work.  The whole input (16 MB/core = 128KB
            # per partition) is SBUF-resident: every x tile gets its own
            # tag so load triggers never carry compute-gated WAR waits --
            # the sync queue is pure loads, streaming at port rate.
            # Stores ride the idle gpsimd SWDGE queue, one round behind.
            NCH = F // CW  # 8 chunks
            xts = {}  # pair -> [xt per t]
            t8s = {}  # pair -> [t8 per t]
            Bs = {}  # chunk -> current B psum tile
            store_q = []

            # loads in DIAGONAL (consumption) order: tile (p, t) is first
            # read at round 2p+t by chunk 2p, so emit loads sorted by that
            # round -- pair-major order would land every t>=2 tile a few us
            # after its consumer and stall the DVE once per round.
            for p in range(NP):
                xts[p] = [
                    pool.tile(
                        [128, PW], f32, tag=f"x{p}_{t}", bufs=1, name=f"x_{p}_{t}"
                    )
                    for t in range(T)
                ]
                t8s[p] = [
                    pool.tile(
                        [128, PW], fp8, tag=f"t8_{p}_{t}", bufs=1,
                        name=f"t8_{p}_{t}",
                    )
                    for t in range(T)
                ]
            for p, t in sorted(
                ((p, t) for p in range(NP) for t in range(T)),
                key=lambda pt: (2 * pt[0] + pt[1], pt[1]),
            ):
                xt = xts[p][t]
                if p == 0 and t == 0 and x0split > 1:
                    wsub = PW // x0split
                    for k in range(x0split):
                        nc.sync.dma_start(
                            out=xt[:, k * wsub : (k + 1) * wsub],
                            in_=x[0][:, k * wsub : (k + 1) * wsub],
                        )
                else:
                    nc.sync.dma_start(out=xt, in_=x[t][:, p * PW : (p + 1) * PW])

            def emit_step(j, t):
                p, ci = j // 2, j % 2
                sl = slice(ci * CW, (ci + 1) * CW)
                if t == 0:
                    u = xts[p][0][:, sl]
                else:
                    u = pool.tile(
                        [128, CW], f32, tag="u", bufs=ubufs, name=f"u_{j}_{t}"
                    )
                    nc.vector.scalar_tensor_tensor(
                        u, Bs[j], 0.0, xts[p][t][:, sl], A.add, A.add
                    )
                if t < T - 1:
                    # reset setup: B = 0.5*u (Act) then B += -0.5*s (PE).
                    # Copy emitted before cmp: both only need u, run on
                    # different engines concurrently.
                    Bn = ppool.tile(
                        [128, CW], f32, tag=f"B{j % NBT}", bufs=1, name=f"B{j}_{t}"
                    )
                    nc.scalar.activation(Bn, u, AF.Copy, 0.0, 0.5)
                    nc.vector.tensor_scalar(t8s[p][t][:, sl], u, 1.0, None, A.is_ge)
                    for h in range(CW // 512):
                        hs = slice(h * 512, (h + 1) * 512)
                        nc.tensor.matmul(
                            Bn[:, hs],
                            wt,
                            t8s[p][t][:, sl][:, hs],
                            start=False,
                            stop=True,
                            skip_group_check=True,
                        )
                    Bs[j] = Bn
                elif j == NCH - 1:
                    # final chunk: fine-grained sign+store tail
                    nq = max(CW // tailsub, 1)
                    q = CW // nq
                    for k in range(nq):
                        usub = slice(k * q, (k + 1) * q)
                        ysub = slice(
                            p * PW + ci * CW + k * q, p * PW + ci * CW + (k + 1) * q
                        )
                        nc.scalar.activation(
                            t8s[p][3][:, sl][:, usub], u[:, usub], AF.Sign,
                            bias=bias,
                        )
                        nc.sync.dma_start(
                            out=y[3][:, ysub], in_=t8s[p][3][:, sl][:, usub]
                        )
                else:
                    nc.scalar.activation(t8s[p][3][:, sl], u, AF.Sign, bias=bias)

            for r in range(NCH + T - 1):
                # flush due stores (t8 of (t, pair p) complete after round
                # 2p+1+t) on the store engine's own queue
                while store_q and store_q[0][0] <= r:
                    _, t, p = store_q.pop(0)
                    if not (t == 3 and p == NP - 1):
                        st.dma_start(
                            out=y[t][:, p * PW : (p + 1) * PW], in_=t8s[p][t]
                        )
                    else:
                        # chunk NCH-1's t3 slice already stored fine-grained
                        st.dma_start(
                            out=y[3][:, p * PW : p * PW + CW], in_=t8s[p][3][:, 0:CW]
                        )
                for j in range(NCH):
                    t = r - j
                    if 0 <= t < T:
                        emit_step(j, t)
                        if j % 2 == 1:  # pair's second chunk passed step t
                            store_q.append((r + 1, t, j // 2))
            while store_q:
                _, t, p = store_q.pop(0)
                if not (t == 3 and p == NP - 1):
                    st.dma_start(out=y[t][:, p * PW : (p + 1) * PW], in_=t8s[p][t])
                else:
                    st.dma_start(
                        out=y[3][:, p * PW : p * PW + CW], in_=t8s[p][3][:, 0:CW]
                    )

    nc.compile()
    return nc


def _build_sign():
    """Fallback: previous SignFlow scheme (~75.4us). See git history of the
    docstring for details; kept for A/B via LIF_SCHEME=sign."""
    import concourse.bacc as bacc
    import concourse.mybir as mybir
    from concourse import tile

    f32 = mybir.dt.float32
    i8 = mybir.dt.int8
    A = mybir.AluOpType
    AF = mybir.ActivationFunctionType

    W = min(int(os.environ.get("LIF_W", "2048")), F)
    CW = min(int(os.environ.get("LIF_CW", str(W))), W)
    NCH = F // W
    SUB = W // CW
    NCC = F // CW
    assert F % W == 0 and W % CW == 0

    nc = bacc.Bacc("TRN2", target_bir_lowering=False, debug=False, num_devices=NCORES)
    x = nc.dram_tensor("x", [T, 128, F], f32, kind="ExternalInput").ap()
    y = nc.dram_tensor("y", [T, 128, F], i8, kind="ExternalOutput").ap()

    xbufs = int(os.environ.get("LIF_XBUFS", "6"))
    ubufs = int(os.environ.get("LIF_UBUFS", "6"))
    tbufs = int(os.environ.get("LIF_TBUFS", "2"))

    with tile.TileContext(nc) as tc:
        with tc.tile_pool(name="p", bufs=xbufs) as pool:
            bias = pool.tile([128, 1], f32, tag="bias", bufs=1)
            warm = pool.tile([128, 1], i8, tag="warm", bufs=1)
            nc.vector.memset(bias, -C_THRESH)
            nc.scalar.activation(warm, bias, AF.Sign, bias=bias)

            W0 = min(int(os.environ.get("LIF_W0", str(CW))), CW)
            nvs = {}
            store_pending = None
            for t in range(T):
                xs = {}
                if t == 0 and W0 < CW:
                    t8 = pool.tile([128, F], i8, tag="t8", bufs=tbufs)
                    for j in range(NCC):
                        nvs[j] = pool.tile(
                            [128, CW], f32, tag=f"nv{j}", bufs=2, name=f"nv0_{j}"
                        )
                    for k in range(F // W0):
                        xt = pool.tile([128, W0], f32, tag="x0", bufs=6)
                        nc.sync.dma_start(out=xt, in_=x[0][:, k * W0 : (k + 1) * W0])
                        sl = slice(k * W0, (k + 1) * W0)
                        nc.scalar.activation(t8[:, sl], xt, AF.Sign, bias=bias)
                        j = (k * W0) // CW
                        nsub = slice(k * W0 - j * CW, (k + 1) * W0 - j * CW)
                        nc.vector.scalar_tensor_tensor(
                            nvs[j][:, nsub], t8[:, sl], 0.0, xt, A.max, A.subtract
                        )
                    nc.sync.dma_start(out=y[0], in_=t8)
                    continue
                for i in range(NCH):
                    xt = pool.tile([128, W], f32, tag="x")
                    nc.sync.dma_start(out=xt, in_=x[t][:, i * W : (i + 1) * W])
                    xs[i] = xt

                if store_pending is not None:
                    pt, pt8 = store_pending
                    nc.sync.dma_start(out=y[pt], in_=pt8)
                    store_pending = None

                t8 = pool.tile([128, F], i8, tag="t8", bufs=tbufs)
                pending = None

                def emit_reset(j, u):
                    sl = slice(j * CW, (j + 1) * CW)
                    nv = pool.tile([128, CW], f32, tag=f"nv{j}", bufs=2)
                    nc.vector.scalar_tensor_tensor(
                        nv, t8[:, sl], 0.0, u, A.max, A.subtract
                    )
                    nvs[j] = nv

                for j in range(NCC):
                    sl = slice(j * CW, (j + 1) * CW)
                    xsl = xs[j // SUB][:, (j % SUB) * CW : (j % SUB + 1) * CW]
                    if t == T - 1 and j == NCC - 1:
                        u = pool.tile([128, CW], f32, tag="u", bufs=ubufs)
                        nq = 4
                        q = CW // nq
                        for k in range(nq):
                            usub = slice(k * q, (k + 1) * q)
                            ysub = slice(j * CW + k * q, j * CW + (k + 1) * q)
                            nc.vector.scalar_tensor_tensor(
                                u[:, usub], nvs[j][:, usub], -0.5,
                                xsl[:, usub], A.mult, A.add,
                            )
                            nc.scalar.activation(
                                t8[:, ysub], u[:, usub], AF.Sign, bias=bias
                            )
                            nc.sync.dma_start(out=y[t][:, ysub], in_=t8[:, ysub])
                        continue
                    if t == 0:
                        u = xsl
                    else:
                        u = pool.tile([128, CW], f32, tag="u", bufs=ubufs)
                        nc.vector.scalar_tensor_tensor(
                            u, nvs[j], -0.5, xsl, A.mult, A.add
                        )
                    if t == 0 and j < 2:
                        nc.vector.tensor_scalar(t8[:, sl], u, 1.0, None, A.is_ge)
                    else:
                        nc.scalar.activation(t8[:, sl], u, AF.Sign, bias=bias)
                    if t < T - 1:
                        if pending is not None:
                            emit_reset(*pending)
                        pending = (j, u)
                    else:
                        nc.sync.dma_start(out=y[t][:, sl], in_=t8[:, sl])
                if pending is not None:
                    emit_reset(*pending)

                if t < T - 1:
                    store_pending = (t, t8)
            if store_pending is not None:
                pt, pt8 = store_pending
                nc.sync.dma_start(out=y[pt], in_=pt8)

    nc.compile()
    return nc


def _get_nc():
    if "nc" not in _CACHE:
        scheme = os.environ.get("LIF_SCHEME", "psum")
        _CACHE["scheme"] = scheme
        _CACHE["nc"] = _build_sign() if scheme == "sign" else _build_psum()
    return _CACHE["nc"]


def kernel(x: np.ndarray) -> np.ndarray:
    global LAST_EXEC_NS, LAST_TRACE
    from concourse.bass_utils import run_bass_kernel_spmd

    x = np.ascontiguousarray(np.asarray(x), dtype=np.float32)
    assert x.shape == (T * B, C, 32, 32), x.shape
    xv = x.reshape(T, B, C, HW)

    nc = _get_nc()
    scheme = _CACHE.get("scheme", "psum")

    wI = None
    if scheme != "sign":
        import ml_dtypes

        wI = (np.eye(128, dtype=np.float32) * -0.5).astype(ml_dtypes.bfloat16)

    in_maps = []
    for m in range(NCORES):
        shard = np.ascontiguousarray(xv[:, m * BLOC : (m + 1) * BLOC]).reshape(
            T, 128, F
        )
        im = {"x": shard}
        if wI is not None:
            im["w"] = wI
        in_maps.append(im)

    trace = os.environ.get("LIF_TRACE") == "1"
    res = run_bass_kernel_spmd(nc, in_maps, core_ids=list(range(NCORES)), trace=trace)
    LAST_EXEC_NS = res.exec_time_ns
    if res.instructions_and_trace is not None:
        LAST_TRACE = res.instructions_and_trace[1]

    out = np.empty((T, B, C, HW), dtype=np.float32)
    for m in range(NCORES):
        raw = np.asarray(res.results[m]["y"])
        if scheme == "sign":
            sp = raw.view(np.int8) == 1
        else:
            # fp8e4 bytes: 1.0 = 0x38 (spike); 0x00 / 0xB8 (-1.0) = no spike
            sp = raw.view(np.uint8) == 0x38
        out[:, m * BLOC : (m + 1) * BLOC] = sp.astype(np.float32).reshape(
            T, BLOC, C, HW
        )
    return out.reshape(T * B, C, 32, 32)
